# revision 27
# baseline (speedup 1.0000x reference)
"""Trainium2 Bass kernel for a dense transformer block (B=4, T=1024, C=1024, H=16).

Sharding: 2 cores per batch element (8 cores / 4 batches). Each core computes
K/V (+LN1) for its full batch but only 4 of the 8 query blocks of 128 rows.
Query blocks are interleaved ({7,4,3,0} on even cores, {6,5,2,1} on odd) so the
causal-attention work is balanced; the compiled program is identical on every
core (SPMD) - per-core behaviour comes only from input data (x slice, gathered
query rows, causal-mask tiles).

v2 restructure vs baseline:
- LN gamma/beta folded into wq/wk/wv/w1 (+bias terms) on the host; bv folded
  through wo into bo. LN emits pure (x-m)*rsqrt(var+eps) (Rsqrt activation).
- Transposes batched 8-per-PSUM-bank with a single DVE evacuation each.
- xq layernormed first so the Q projection overlaps the xb layernorms; QKV
  PSUM evacuations moved to ScalarE (idle in that window).
- Attention pipelined per (slot, head-group): scores for 8 heads -> 2 PSUM
  banks, ONE wide exp [128,1024] per k-block, binary bf16 masks applied
  post-exp on DVE, AV h8-outer/kb-inner, denominators via one wide DVE
  reciprocal + GpSimd partition_broadcast.
- w1/w2 DMA prefetched right after attention.
"""
import os
import sys

for _p in ("/opt/trn_rl_repo", "/root/.axon_site/_ro/trn_rl_repo"):
    if os.path.isdir(_p) and _p not in sys.path:
        sys.path.insert(0, _p)

from contextlib import ExitStack

import ml_dtypes
import numpy as np

import concourse.bass as bass
import concourse.tile as tile
from concourse import library_config, mybir
from concourse.bass_utils import run_bass_kernel_spmd
from concourse.masks import make_identity

F32 = mybir.dt.float32
BF16 = mybir.dt.bfloat16
AF = mybir.ActivationFunctionType
OP = mybir.AluOpType

B, T, C, H, D = 4, 1024, 1024, 16, 64
F = 4 * C                       # MLP hidden
NB = T // 128                   # 8 row blocks per batch
NSLOT = 4                       # query blocks per core
KMAX = [8, 6, 4, 2]             # k-blocks computed per slot (max over both cores)
QBLOCKS = [[7, 4, 3, 0], [6, 5, 2, 1]]  # global q-block per slot, by core parity
# (slot, kb) pairs that need a data mask (kb below min over parities: always allow)
MASKED = [(0, 6), (0, 7), (1, 4), (1, 5), (2, 2), (2, 3), (3, 0), (3, 1)]
EPS = 1e-5


def build_nc():
    nc = bass.Bass("TRN2")

    # ---- DRAM I/O ----------------------------------------------------------
    xb = nc.dram_tensor("xb", [T, C], F32, kind="ExternalInput")     # full batch rows
    xq = nc.dram_tensor("xq", [512, C], F32, kind="ExternalInput")   # gathered q rows
    masks = nc.dram_tensor("masks", [8, 128, 1024], BF16, kind="ExternalInput")
    wq = nc.dram_tensor("wq", [C, C], BF16, kind="ExternalInput")
    wk = nc.dram_tensor("wk", [C, C], BF16, kind="ExternalInput")
    wv = nc.dram_tensor("wv", [C, C], BF16, kind="ExternalInput")
    wo = nc.dram_tensor("wo", [C, C], BF16, kind="ExternalInput")
    w1 = nc.dram_tensor("w1", [C, F], BF16, kind="ExternalInput")
    w2 = nc.dram_tensor("w2", [F, C], BF16, kind="ExternalInput")
    bq = nc.dram_tensor("bq", [C], F32, kind="ExternalInput")
    bk = nc.dram_tensor("bk", [C], F32, kind="ExternalInput")
    bo = nc.dram_tensor("bo", [C], F32, kind="ExternalInput")
    b1 = nc.dram_tensor("b1", [F], F32, kind="ExternalInput")
    b2 = nc.dram_tensor("b2", [C], F32, kind="ExternalInput")
    out = nc.dram_tensor("out", [512, C], F32, kind="ExternalOutput")

    with tile.TileContext(nc) as tc, ExitStack() as ctx:
        consts = ctx.enter_context(tc.tile_pool(name="consts", bufs=1))
        small = ctx.enter_context(tc.tile_pool(name="small", bufs=4))

        # ---- constants -----------------------------------------------------
        ident = consts.tile([128, 128], BF16, tag="ident", name="ident")
        make_identity(nc, ident)
        ones_row = consts.tile([1, 64], BF16, tag="ones_row", name="ones_row")
        nc.vector.memset(ones_row, 1.0)
        eps_col = consts.tile([128, 1], F32, tag="eps", name="eps")
        nc.vector.memset(eps_col, EPS)

        def load_cols(dram, nblk, tag):
            t = consts.tile([128, nblk], F32, tag=tag)
            nc.sync.dma_start(out=t, in_=dram.rearrange("(a p) -> p a", p=128))
            return t

        # free-dim biases, broadcast across partitions via DMA
        def load_bcast(dram, tag):
            t = consts.tile([128, C], F32, tag=tag)
            nc.sync.dma_start(
                out=t,
                in_=dram.rearrange("(one c) -> one c", one=1).partition_broadcast(128))
            return t

        xmid = consts.tile([128, 4, C], F32, tag="xmid", name="xmid")

        att_ctx = ExitStack()
        p_att = att_ctx.enter_context(tc.tile_pool(name="p_att", bufs=1))
        p_w = att_ctx.enter_context(tc.tile_pool(name="p_w", bufs=2))

        # per-slot xq chunks so LN stats start after the first 0.5 MB lands
        xq_sb = p_att.tile([128, 4, C], F32, tag="xq", name="xq")
        for j in range(NSLOT):
            nc.sync.dma_start(
                out=xq_sb[:, j, :],
                in_=xq[j * 128:(j + 1) * 128, :])
        bqc = load_cols(bq, 8, "bqc")
        bkc = load_cols(bk, 8, "bkc")

        def ln_stats(pool, x_aps, tagp):
            """Batched LN stats for a list of row-blocks: returns (mv, rstd)
            with mv [128, n, 2] (mean, var) and rstd [128, n] = 1/sqrt(var+eps).
            One Sqrt activation + one wide DVE reciprocal for the whole group."""
            n = len(x_aps)
            mv = pool.tile([128, n, 2], F32, tag=f"ln_mv_{tagp}", name="ln_mv")
            for i, x_ap in enumerate(x_aps):
                stats = pool.tile([128, 2, 6], F32, tag="ln_stats", name="ln_stats",
                                  bufs=3)
                for s in range(2):
                    nc.vector.bn_stats(out=stats[:, s, :],
                                       in_=x_ap[:, s * 512:(s + 1) * 512])
                nc.vector.bn_aggr(out=mv[:, i, :], in_=stats)
            # rstd = Exp(-0.5*Ln(var+eps)): keeps the whole kernel on the
            # natural_log_exp activation table set (no Sqrt set load before
            # the attention exps, no DVE reciprocal)
            lnv = pool.tile([128, n], F32, tag=f"ln_lnv_{tagp}", name="ln_lnv")
            nc.scalar.activation(out=lnv, in_=mv[:, :, 1], func=AF.Ln, bias=eps_col)
            rstd = pool.tile([128, n], F32, tag=f"ln_rstd_{tagp}", name="ln_rstd")
            nc.scalar.activation(out=rstd, in_=lnv, func=AF.Exp, scale=-0.5)
            return mv, rstd

        def ln_apply(x_ap, mv, rstd, i, h_out_ap):
            nc.vector.tensor_scalar(out=h_out_ap, in0=x_ap, scalar1=mv[:, i, 0:1],
                                    scalar2=rstd[:, i:i + 1],
                                    op0=OP.subtract, op1=OP.mult)

        def transpose_block(ps_pool, pool, h_rows, hT_all, rcol):
            """8 PE transposes into one PSUM bank; single DVE evacuation."""
            tp8 = ps_pool.tile([128, 8, 128], BF16, tag="tp8", name="tp8")
            for c in range(8):
                nc.tensor.transpose(tp8[:, c, :], h_rows[:, c * 128:(c + 1) * 128], ident)
            nc.vector.tensor_copy(out=hT_all[:, :, rcol:rcol + 128], in_=tp8)

        # ==== phase 1+2: LN1 + transpose ====================================
        # xq first (gates Q projection), then xb rows (gate K/V).
        h1_ctx = ExitStack()
        p_h1 = h1_ctx.enter_context(tc.tile_pool(name="p_h1", bufs=1))
        h1T = p_h1.tile([128, 8, T], BF16, tag="h1T", name="h1T")
        hqT = p_h1.tile([128, 8, 512], BF16, tag="hqT", name="hqT")

        ph1s = h1_ctx.enter_context(tc.tile_pool(name="p_h1s", bufs=3))
        ps_t = h1_ctx.enter_context(tc.tile_pool(name="ps_t", bufs=3, space="PSUM"))

        mv_q, rstd_q = ln_stats(ph1s, [xq_sb[:, j, :] for j in range(NSLOT)], "q")
        for j in range(NSLOT):
            h_rows = ph1s.tile([128, C], BF16, tag="h_rows", name="h_rows")
            ln_apply(xq_sb[:, j, :], mv_q, rstd_q, j, h_rows)
            transpose_block(ps_t, ph1s, h_rows, hqT, j * 128)

        qT = p_att.tile([128, 8, 512], BF16, tag="qT", name="qT")
        kT = p_att.tile([128, 8, T], BF16, tag="kT", name="kT")
        vaug = p_att.tile([128, 8, 16, 65], BF16, tag="vaug", name="vaug")
        yT = p_att.tile([128, 8, 512], BF16, tag="yT", name="yT")

        wq_sb = p_w.tile([128, 8, C], BF16, tag="wslab", name="wslab")
        nc.sync.dma_start(out=wq_sb, in_=wq.rearrange("(a p) c -> p a c", p=128))
        wk_sb = p_w.tile([128, 8, C], BF16, tag="wslab", name="wslab")
        nc.sync.dma_start(out=wk_sb, in_=wk.rearrange("(a p) c -> p a c", p=128))

        ps_mm = h1_ctx.enter_context(tc.tile_pool(name="ps_mm", bufs=4, space="PSUM"))

        # Q^T from hqT -> [C, 512] (evacuate + bias on ScalarE)
        for co in range(8):
            ps = ps_mm.tile([128, 512], F32, tag="mm", name="mm")
            for ci in range(8):
                nc.tensor.matmul(ps, lhsT=wq_sb[:, ci, co * 128:(co + 1) * 128],
                                 rhs=hqT[:, ci, :], start=(ci == 0), stop=(ci == 7))
            nc.scalar.activation(out=qT[:, co, :], in_=ps, func=AF.Identity,
                                 bias=bqc[:, co:co + 1])

        # LN1 of the full batch rows (DVE work overlaps the Q matmuls above)
        x_ts = []
        for r in range(NB):
            x_t = ph1s.tile([128, C], F32, tag="x_t", name="x_t", bufs=NB)
            nc.sync.dma_start(out=x_t, in_=xb[r * 128:(r + 1) * 128, :])
            x_ts.append(x_t)
        mv_b, rstd_b = ln_stats(ph1s, x_ts, "b")
        for r in range(NB):
            h_rows = ph1s.tile([128, C], BF16, tag="h_rows", name="h_rows")
            ln_apply(x_ts[r], mv_b, rstd_b, r, h_rows)
            transpose_block(ps_t, ph1s, h_rows, h1T, r * 128)

        # K^T from h1T -> [C, T]
        for co in range(8):
            for nt in range(2):
                ps = ps_mm.tile([128, 512], F32, tag="mm", name="mm")
                for ci in range(8):
                    nc.tensor.matmul(
                        ps, lhsT=wk_sb[:, ci, co * 128:(co + 1) * 128],
                        rhs=h1T[:, ci, nt * 512:(nt + 1) * 512],
                        start=(ci == 0), stop=(ci == 7))
                nc.scalar.activation(out=kT[:, co, nt * 512:(nt + 1) * 512], in_=ps,
                                     func=AF.Identity, bias=bkc[:, co:co + 1])

        wv_sb = p_w.tile([128, 8, C], BF16, tag="wslab", name="wslab")
        nc.sync.dma_start(out=wv_sb, in_=wv.rearrange("(a p) c -> p a c", p=128))
        # V rows (bias folded into bo on host), interleaved with ones column
        nc.vector.memset(vaug[:, :, :, 64:65], 1.0)
        for tk in range(8):
            for nt in range(2):
                ps = ps_mm.tile([128, 512], F32, tag="mm", name="mm")
                for ci in range(8):
                    nc.tensor.matmul(
                        ps, lhsT=h1T[:, ci, tk * 128:(tk + 1) * 128],
                        rhs=wv_sb[:, ci, nt * 512:(nt + 1) * 512],
                        start=(ci == 0), stop=(ci == 7))
                nc.scalar.activation(
                    out=vaug[:, tk, nt * 8:(nt + 1) * 8, 0:64],
                    in_=ps.rearrange("p (h d) -> p h d", d=64), func=AF.Identity)

        wo_sb = p_w.tile([128, 8, C], BF16, tag="wslab", name="wslab")
        nc.sync.dma_start(out=wo_sb, in_=wo.rearrange("(a p) c -> p a c", p=128))

        mask_sb = p_att.tile([128, 8, 1024], BF16, tag="masks", name="masks")
        nc.sync.dma_start(out=mask_sb, in_=masks.rearrange("m p q -> p m q"))
        BO = load_bcast(bo, "BO")

        h1_ctx.close()

        # ==== phase 4: attention (pipelined over (slot, head-group)) ========
        mask_idx = {sk: i for i, sk in enumerate(MASKED)}
        groups = [(j, hg) for j in range(NSLOT) for hg in range(2)]

        with tc.tile_pool(name="p_exp", bufs=1) as pexp, \
             tc.tile_pool(name="p_dn", bufs=2) as pdn, \
             tc.tile_pool(name="ps_s", bufs=2, space="PSUM") as ps_s, \
             tc.tile_pool(name="ps_y", bufs=2, space="PSUM") as ps_y:

            def emit_scores(j, hg, kb):
                s_ps = ps_s.tile([128, 1024], F32, tag="s_ps", name="s_ps")
                for p in range(4):
                    hp = 4 * hg + p
                    for hh in range(2):
                        fl = 4 * hh + p
                        nc.tensor.matmul(
                            s_ps[:, fl * 128:(fl + 1) * 128],
                            lhsT=kT[hh * 64:(hh + 1) * 64, hp, kb * 128:(kb + 1) * 128],
                            rhs=qT[hh * 64:(hh + 1) * 64, hp, j * 128:(j + 1) * 128],
                            start=True, stop=True, tile_position=(64 * hh, 0))
                return s_ps

            def emit_exp(j, hg, kb, s_ps):
                expS = pexp.tile([128, 1024], BF16, tag="expS", name="expS", bufs=16)
                nc.scalar.activation(out=expS, in_=s_ps, func=AF.Exp, scale=0.125)
                if (j, kb) in mask_idx:
                    mi = mask_idx[(j, kb)]
                    nc.vector.tensor_mul(out=expS, in0=expS, in1=mask_sb[:, mi, :])
                return expS

            def av_mms(j, hg, expS_list, yaug):
                km = KMAX[j]
                mms = []
                for h8 in range(8):
                    fl = 4 * (h8 % 2) + h8 // 2
                    for kb in range(km):
                        mms.append((yaug[:, h8 * 128:(h8 + 1) * 128],
                                    (kb, 8 * hg + h8),
                                    expS_list[kb][:, fl * 128:(fl + 1) * 128],
                                    kb == 0, kb == km - 1))
                return mms

            def emit_denorm_act(yaug):
                # 1/d via Exp(-Ln(d)) on ScalarE: a [1,N] DVE reciprocal runs
                # single-lane at ~6.4ns/elem (6.5us/call); the two wide ACT
                # calls cost 2.3us and Ln+Exp share one activation table set.
                lnd = pdn.tile([1, 1024], F32, tag="lnd", name="lnd")
                nc.scalar.activation(out=lnd, in_=yaug[64:65, :], func=AF.Ln)
                rbf = pdn.tile([1, 1024], BF16, tag="rbf", name="rbf")
                nc.scalar.activation(out=rbf, in_=lnd, func=AF.Exp, scale=-1.0)
                return rbf

            def emit_denorm_pe(j, hg, yaug, rbf):
                # K=1 ones-matmul broadcast across 64 partitions; the PSUM
                # tile reuses the scores ring (same shape/tag) to stay in the
                # 8-bank budget. Emitted well after the ACT part so the PE
                # stream never blocks waiting for rbf.
                rb_ps = ps_s.tile([128, 1024], F32, tag="s_ps", name="rb_ps")
                for nt in range(2):
                    nc.tensor.matmul(rb_ps[0:64, nt * 512:(nt + 1) * 512],
                                     lhsT=ones_row, rhs=rbf[:, nt * 512:(nt + 1) * 512],
                                     start=True, stop=True)
                rb_sb = pdn.tile([64, 1024], F32, tag="rb_sb", name="rb_sb")
                nc.vector.tensor_copy(out=rb_sb, in_=rb_ps[0:64, :])
                ya = yaug.rearrange("p (hp two q) -> p hp two q", two=2, q=128)
                rb = rb_sb.rearrange("p (hp two q) -> p hp two q", two=2, q=128)
                for par in range(2):
                    nc.vector.tensor_mul(
                        out=yT[par * 64:(par + 1) * 64, 4 * hg:4 * hg + 4,
                               j * 128:(j + 1) * 128],
                        in0=ya[0:64, :, par, :], in1=rb[0:64, :, par, :])

            def emit_av(mms):
                for o, (kb, h), e, st, sp in mms:
                    nc.tensor.matmul(o, lhsT=vaug[:, kb, h, :], rhs=e,
                                     start=st, stop=sp)

            prev = None  # (j, hg, pending AV mm list, yaug)
            for j, hg in groups:
                km = KMAX[j]
                expS_list = []
                # front-load prev group's AV matmuls into the first half of
                # this group's k-blocks; the denorm chain (ACT lnd/exp -> PE
                # broadcast -> DVE copy/muls) is then emitted mid-group so it
                # completes during the second half instead of stalling PE at
                # the group boundary (which re-throttles the HAM clock).
                nch = max(1, km // 2)
                if prev is not None:
                    pmms = prev[2]
                    csz = -(-len(pmms) // nch)
                    chunks = [pmms[i:i + csz] for i in range(0, len(pmms), csz)]
                else:
                    chunks = []

                rbf_prev = None
                for kb in range(km):
                    s_ps = emit_scores(j, hg, kb)
                    if kb < len(chunks):
                        emit_av(chunks[kb])
                    expS_list.append(emit_exp(j, hg, kb, s_ps))
                    if prev is not None and kb == nch:
                        rbf_prev = emit_denorm_act(prev[3])
                if prev is not None:
                    emit_denorm_pe(prev[0], prev[1], prev[3], rbf_prev)

                yaug = ps_y.tile([65, 1024], F32, tag="yaug", name="yaug")
                prev = (j, hg, av_mms(j, hg, expS_list, yaug), yaug)

            # drain the last group
            emit_av(prev[2])
            rbf_last = emit_denorm_act(prev[3])
            emit_denorm_pe(prev[0], prev[1], prev[3], rbf_last)

        # ==== phase 5: output projection + residual ========================
        with tc.tile_pool(name="ps_pr", bufs=4, space="PSUM") as ps_pr:
            for j in range(NSLOT):
                for nt in range(2):
                    ps = ps_pr.tile([128, 512], F32, tag="prj", name="prj")
                    for ci in range(8):
                        nc.tensor.matmul(
                            ps, lhsT=yT[:, ci, j * 128:(j + 1) * 128],
                            rhs=wo_sb[:, ci, nt * 512:(nt + 1) * 512],
                            start=(ci == 0), stop=(ci == 7))
                    t1 = small.tile([128, 512], F32, tag="prt", name="prt", bufs=2)
                    nc.vector.tensor_add(t1, ps, BO[:, nt * 512:(nt + 1) * 512])
                    nc.vector.tensor_add(
                        xmid[:, j, nt * 512:(nt + 1) * 512], t1,
                        xq_sb[:, j, nt * 512:(nt + 1) * 512])

        att_ctx.close()

        p_mlp = ctx.enter_context(tc.tile_pool(name="p_mlp", bufs=1))
        pw1 = ctx.enter_context(tc.tile_pool(name="p_w1", bufs=2))
        pw2 = ctx.enter_context(tc.tile_pool(name="p_w2", bufs=2))

        b1c = load_cols(b1, 32, "b1c")
        B2 = load_bcast(b2, "B2")

        # prefetch MLP weights (DMA runs under LN2/MLP1 compute)
        w1c = [pw1.tile([128, 8, C], BF16, tag="w1c", name="w1c") for _ in range(2)]
        for chunk in range(2):
            nc.sync.dma_start(
                out=w1c[chunk], in_=w1[:, chunk * C:(chunk + 1) * C]
                .rearrange("(a p) c -> p a c", p=128))
        w2h = [pw2.tile([128, 16, C], BF16, tag="w2h", name="w2h") for _ in range(2)]
        for half in range(2):
            nc.sync.dma_start(
                out=w2h[half], in_=w2[half * 2048:(half + 1) * 2048, :]
                .rearrange("(a p) c -> p a c", p=128))

        # ==== phase 6: LN2 + transpose -> h2T [C, 512] bf16 =================
        h2T = p_mlp.tile([128, 8, 512], BF16, tag="h2T", name="h2T")
        with tc.tile_pool(name="p_h2s", bufs=2) as ph2s, \
             tc.tile_pool(name="ps_t2", bufs=2, space="PSUM") as ps_t2:
            mv_2, rstd_2 = ln_stats(ph2s, [xmid[:, j, :] for j in range(NSLOT)], "2")
            for j in range(NSLOT):
                h2_rows = ph2s.tile([128, C], BF16, tag="h2_rows", name="h2_rows")
                ln_apply(xmid[:, j, :], mv_2, rstd_2, j, h2_rows)
                transpose_block(ps_t2, ph2s, h2_rows, h2T, j * 128)

        # ==== phase 7: MLP1 + gelu -> mT [F, 512] bf16 ======================
        mT = p_mlp.tile([128, 32, 512], BF16, tag="mT", name="mT")
        with tc.tile_pool(name="ps_m1", bufs=4, space="PSUM") as ps_m1:
            for chunk in range(4):
                if chunk >= 2:
                    wc = pw1.tile([128, 8, C], BF16, tag="w1c", name="w1c")
                    nc.sync.dma_start(
                        out=wc, in_=w1[:, chunk * C:(chunk + 1) * C]
                        .rearrange("(a p) c -> p a c", p=128))
                else:
                    wc = w1c[chunk]
                for co8 in range(8):
                    co = chunk * 8 + co8
                    ps = ps_m1.tile([128, 512], F32, tag="m1", name="m1")
                    for ci in range(8):
                        nc.tensor.matmul(
                            ps, lhsT=wc[:, ci, co8 * 128:(co8 + 1) * 128],
                            rhs=h2T[:, ci, :], start=(ci == 0), stop=(ci == 7))
                    nc.scalar.activation(out=mT[:, co, :], in_=ps, func=AF.Gelu,
                                         bias=b1c[:, co:co + 1])

        # ==== phase 8: MLP2 + residual -> out ===============================
        with tc.tile_pool(name="p_out", bufs=2) as pout, \
             tc.tile_pool(name="ps_m2", bufs=8, space="PSUM") as ps_m2:
            pss = [ps_m2.tile([128, 512], F32, tag="m2", name="m2")
                   for _ in range(8)]
            for half in range(2):
                for j in range(NSLOT):
                    for nt in range(2):
                        ps = pss[j * 2 + nt]
                        for ka in range(16):
                            ki = half * 16 + ka
                            nc.tensor.matmul(
                                ps, lhsT=mT[:, ki, j * 128:(j + 1) * 128],
                                rhs=w2h[half][:, ka, nt * 512:(nt + 1) * 512],
                                start=(ki == 0), stop=(ki == 31))
            for j in range(NSLOT):
                o_sb = pout.tile([128, C], F32, tag="o_sb", name="o_sb")
                for nt in range(2):
                    t1 = small.tile([128, 512], F32, tag="ot", name="ot", bufs=2)
                    nc.vector.tensor_add(t1, pss[j * 2 + nt],
                                         B2[:, nt * 512:(nt + 1) * 512])
                    nc.vector.tensor_add(
                        o_sb[:, nt * 512:(nt + 1) * 512], t1,
                        xmid[:, j, nt * 512:(nt + 1) * 512])
                nc.sync.dma_start(out=out[j * 128:(j + 1) * 128, :], in_=o_sb)

    _split_excess_waits(nc)
    return nc


def _split_excess_waits(nc, max_waits=1):
    """walrus rejects engine instructions with >1 sync wait. Hoist excess
    waits onto standalone EventSemaphore (pure-wait) instructions inserted
    just before the offending instruction on the same engine."""
    counter = 0
    for fn in nc.m.functions:
        for bb in fn.blocks:
            insts = bb.instructions
            i = 0
            while i < len(insts):
                inst = insts[i]
                si = getattr(inst, "sync_info", None)
                if os.environ.get("KEEP_DMA_WAITS") and \
                        type(inst).__name__ == "InstDMACopy":
                    i += 1
                    continue
                if (si is not None and si.on_wait
                        and len(si.on_wait) > max_waits):
                    waits = list(si.on_wait)
                    keep, extra = waits[-max_waits:], waits[:-max_waits]
                    for w in extra:
                        ev = mybir.InstEventSemaphore(
                            name=f"splitwait_{counter}", ins=[], outs=[])
                        counter += 1
                        ev.engine = inst.engine
                        ev.bass_nofuse = True
                        ev.sync_info = mybir.SyncInfo(on_wait=[w], on_update=[])
                        nc.register_instruction(ev)
                        insts.insert(i, ev)
                        i += 1
                    inst.sync_info = mybir.SyncInfo(
                        on_wait=keep, on_update=list(si.on_update))
                i += 1


_NC_CACHE = None


def _get_nc():
    global _NC_CACHE
    if _NC_CACHE is None:
        _NC_CACHE = build_nc()
    return _NC_CACHE


def make_masks(parity: int) -> np.ndarray:
    """[8,128,1024] multiplicative bf16 0/1 mask tiles (replicated across the
    8 head-slices) for the MASKED (slot,kb) pairs. Layout [k, q]: keep k<=q."""
    tiles = np.zeros((8, 128, 1024), np.float32)
    tri = (np.arange(128)[:, None] <= np.arange(128)[None, :]).astype(np.float32)
    for i, (slot, kb) in enumerate(MASKED):
        g = QBLOCKS[parity][slot]
        if kb < g:
            tiles[i] = 1.0
        elif kb == g:
            tiles[i] = np.tile(tri, (1, 8))
        else:
            tiles[i] = 0.0
    return tiles.astype(ml_dtypes.bfloat16)


def fold_weights(weights: dict) -> dict:
    """Fold LN gamma/beta into the adjacent projection weights (fp64 on host):
    q = n1 @ (g1*wq) + (bq + b1*wq), same for k; v loses its bias entirely
    (A rows sum to 1 -> bv' routes through wo into bo); ln2 folds into w1."""
    f8 = lambda a: np.asarray(a, np.float64)
    g1, b1g = f8(weights["ln1_g"]), f8(weights["ln1_b"])
    g2, b2g = f8(weights["ln2_g"]), f8(weights["ln2_b"])
    wq, wk, wv, wo = (f8(weights[k]) for k in ("wq", "wk", "wv", "wo"))
    w1, w2 = f8(weights["w1"]), f8(weights["w2"])
    bq, bk, bv, bo = (f8(weights[k]) for k in ("bq", "bk", "bv", "bo"))
    b1, b2 = f8(weights["b1"]), f8(weights["b2"])

    wqf = g1[:, None] * wq
    wkf = g1[:, None] * wk
    wvf = g1[:, None] * wv
    bqf = bq + b1g @ wq
    bkf = bk + b1g @ wk
    bvf = bv + b1g @ wv
    bof = bo + bvf @ wo
    w1f = g2[:, None] * w1
    b1f = b1 + b2g @ w1

    bf = lambda a: np.ascontiguousarray(a.astype(np.float32)).astype(ml_dtypes.bfloat16)
    f32 = lambda a: np.ascontiguousarray(a.astype(np.float32))
    return {
        "wq": bf(wqf), "wk": bf(wkf), "wv": bf(wvf), "wo": bf(wo),
        "w1": bf(w1f), "w2": bf(w2),
        "bq": f32(bqf), "bk": f32(bkf), "bo": f32(bof),
        "b1": f32(b1f), "b2": f32(b2),
    }


def make_in_maps(x: np.ndarray, weights: dict) -> list[dict]:
    f32 = lambda a: np.ascontiguousarray(np.asarray(a, np.float32))
    shared = fold_weights(weights)
    mask_by_parity = [make_masks(0), make_masks(1)]
    in_maps = []
    for core in range(8):
        b, parity = core // 2, core % 2
        qb = QBLOCKS[parity]
        xqg = np.concatenate([x[b, g * 128:(g + 1) * 128, :] for g in qb], axis=0)
        in_maps.append({
            "xb": f32(x[b]), "xq": f32(xqg), "masks": mask_by_parity[parity],
            **shared,
        })
    return in_maps


def assemble_out(results: list[dict]) -> np.ndarray:
    out = np.empty((B, T, C), np.float32)
    for core in range(8):
        b, parity = core // 2, core % 2
        o = np.asarray(results[core]["out"], np.float32)
        for j, g in enumerate(QBLOCKS[parity]):
            out[b, g * 128:(g + 1) * 128, :] = o[j * 128:(j + 1) * 128, :]
    return out


def kernel(**inputs) -> np.ndarray:
    x = np.asarray(inputs["x"], np.float32)
    nc = _get_nc()
    in_maps = make_in_maps(x, inputs)
    res = run_bass_kernel_spmd(nc, in_maps, list(range(8)))
    return assemble_out(res.results)


if __name__ == "__main__":
    _get_nc()
    print("built ok")


# revision 29
# speedup vs baseline: 1.0315x; 1.0315x over previous
"""Trainium2 Bass kernel for a dense transformer block (B=4, T=1024, C=1024, H=16).

Sharding: 2 cores per batch element (8 cores / 4 batches). Each core computes
K/V (+LN1) for its full batch but only 4 of the 8 query blocks of 128 rows.
Query blocks are interleaved ({7,4,3,0} on even cores, {6,5,2,1} on odd) so the
causal-attention work is balanced; the compiled program is identical on every
core (SPMD) - per-core behaviour comes only from input data (x slice, gathered
query rows, causal-mask tiles).

v2 restructure vs baseline:
- LN gamma/beta folded into wq/wk/wv/w1 (+bias terms) on the host; bv folded
  through wo into bo. LN emits pure (x-m)*rsqrt(var+eps) (Rsqrt activation).
- Transposes batched 8-per-PSUM-bank with a single DVE evacuation each.
- xq layernormed first so the Q projection overlaps the xb layernorms; QKV
  PSUM evacuations moved to ScalarE (idle in that window).
- Attention pipelined per (slot, head-group): scores for 8 heads -> 2 PSUM
  banks, ONE wide exp [128,1024] per k-block, binary bf16 masks applied
  post-exp on DVE, AV h8-outer/kb-inner, denominators via one wide DVE
  reciprocal + GpSimd partition_broadcast.
- w1/w2 DMA prefetched right after attention.
"""
import os
import sys

for _p in ("/opt/trn_rl_repo", "/root/.axon_site/_ro/trn_rl_repo"):
    if os.path.isdir(_p) and _p not in sys.path:
        sys.path.insert(0, _p)

from contextlib import ExitStack

import ml_dtypes
import numpy as np

import concourse.bass as bass
import concourse.tile as tile
from concourse import library_config, mybir
from concourse.bass_utils import run_bass_kernel_spmd
from concourse.masks import make_identity

F32 = mybir.dt.float32
BF16 = mybir.dt.bfloat16
AF = mybir.ActivationFunctionType
OP = mybir.AluOpType

B, T, C, H, D = 4, 1024, 1024, 16, 64
F = 4 * C                       # MLP hidden
NB = T // 128                   # 8 row blocks per batch
NSLOT = 4                       # query blocks per core
KMAX = [8, 6, 4, 2]             # k-blocks computed per slot (max over both cores)
QBLOCKS = [[7, 4, 3, 0], [6, 5, 2, 1]]  # global q-block per slot, by core parity
# (slot, kb) pairs that need a data mask (kb below min over parities: always allow)
MASKED = [(0, 6), (0, 7), (1, 4), (1, 5), (2, 2), (2, 3), (3, 0), (3, 1)]
EPS = 1e-5


def build_nc():
    nc = bass.Bass("TRN2")

    # ---- DRAM I/O ----------------------------------------------------------
    xb = nc.dram_tensor("xb", [T, C], BF16, kind="ExternalInput")     # full batch rows
    xq = nc.dram_tensor("xq", [512, C], BF16, kind="ExternalInput")   # gathered q rows
    masks = nc.dram_tensor("masks", [8, 128, 1024], BF16, kind="ExternalInput")
    wq = nc.dram_tensor("wq", [C, C], BF16, kind="ExternalInput")
    wk = nc.dram_tensor("wk", [C, C], BF16, kind="ExternalInput")
    wv = nc.dram_tensor("wv", [C, C], BF16, kind="ExternalInput")
    wo = nc.dram_tensor("wo", [C, C], BF16, kind="ExternalInput")
    w1 = nc.dram_tensor("w1", [C, F], BF16, kind="ExternalInput")
    w2 = nc.dram_tensor("w2", [F, C], BF16, kind="ExternalInput")
    bq = nc.dram_tensor("bq", [C], F32, kind="ExternalInput")
    bk = nc.dram_tensor("bk", [C], F32, kind="ExternalInput")
    bo = nc.dram_tensor("bo", [C], F32, kind="ExternalInput")
    b1 = nc.dram_tensor("b1", [F], F32, kind="ExternalInput")
    b2 = nc.dram_tensor("b2", [C], F32, kind="ExternalInput")
    out = nc.dram_tensor("out", [512, C], F32, kind="ExternalOutput")

    with tile.TileContext(nc) as tc, ExitStack() as ctx:
        consts = ctx.enter_context(tc.tile_pool(name="consts", bufs=1))
        small = ctx.enter_context(tc.tile_pool(name="small", bufs=4))

        # ---- constants -----------------------------------------------------
        ident = consts.tile([128, 128], BF16, tag="ident", name="ident")
        make_identity(nc, ident)
        ones_row = consts.tile([1, 64], BF16, tag="ones_row", name="ones_row")
        nc.vector.memset(ones_row, 1.0)
        eps_col = consts.tile([128, 1], F32, tag="eps", name="eps")
        nc.vector.memset(eps_col, EPS)

        def load_cols(dram, nblk, tag):
            t = consts.tile([128, nblk], F32, tag=tag)
            nc.sync.dma_start(out=t, in_=dram.rearrange("(a p) -> p a", p=128))
            return t

        # free-dim biases, broadcast across partitions via DMA
        def load_bcast(dram, tag):
            t = consts.tile([128, C], F32, tag=tag)
            nc.sync.dma_start(
                out=t,
                in_=dram.rearrange("(one c) -> one c", one=1).partition_broadcast(128))
            return t

        xmid = consts.tile([128, 4, C], F32, tag="xmid", name="xmid")

        att_ctx = ExitStack()
        p_att = att_ctx.enter_context(tc.tile_pool(name="p_att", bufs=1))
        p_w = att_ctx.enter_context(tc.tile_pool(name="p_w", bufs=2))

        # per-slot xq chunks so LN stats start after the first 0.5 MB lands
        xq_sb = p_att.tile([128, 4, C], BF16, tag="xq", name="xq")
        for j in range(NSLOT):
            nc.sync.dma_start(
                out=xq_sb[:, j, :],
                in_=xq[j * 128:(j + 1) * 128, :])
        bqc = load_cols(bq, 8, "bqc")
        bkc = load_cols(bk, 8, "bkc")

        def ln_stats(pool, x_aps, tagp):
            """Batched LN stats for a list of row-blocks: returns (mv, rstd)
            with mv [128, n, 2] (mean, var) and rstd [128, n] = 1/sqrt(var+eps).
            One Sqrt activation + one wide DVE reciprocal for the whole group."""
            n = len(x_aps)
            mv = pool.tile([128, n, 2], F32, tag=f"ln_mv_{tagp}", name="ln_mv")
            for i, x_ap in enumerate(x_aps):
                stats = pool.tile([128, 2, 6], F32, tag="ln_stats", name="ln_stats",
                                  bufs=3)
                for s in range(2):
                    nc.vector.bn_stats(out=stats[:, s, :],
                                       in_=x_ap[:, s * 512:(s + 1) * 512])
                nc.vector.bn_aggr(out=mv[:, i, :], in_=stats)
            std = pool.tile([128, n], F32, tag=f"ln_std_{tagp}", name="ln_std")
            nc.scalar.activation(out=std, in_=mv[:, :, 1], func=AF.Sqrt, bias=eps_col)
            rstd = pool.tile([128, n], F32, tag=f"ln_rstd_{tagp}", name="ln_rstd")
            nc.vector.reciprocal(out=rstd, in_=std)
            return mv, rstd

        def ln_apply(x_ap, mv, rstd, i, h_out_ap):
            nc.vector.tensor_scalar(out=h_out_ap, in0=x_ap, scalar1=mv[:, i, 0:1],
                                    scalar2=rstd[:, i:i + 1],
                                    op0=OP.subtract, op1=OP.mult)

        def transpose_block(ps_pool, pool, h_rows, hT_all, rcol):
            """8 PE transposes into one PSUM bank; single DVE evacuation."""
            tp8 = ps_pool.tile([128, 8, 128], BF16, tag="tp8", name="tp8")
            for c in range(8):
                nc.tensor.transpose(tp8[:, c, :], h_rows[:, c * 128:(c + 1) * 128], ident)
            nc.vector.tensor_copy(out=hT_all[:, :, rcol:rcol + 128], in_=tp8)

        # ==== phase 1+2: LN1 + transpose ====================================
        # xq first (gates Q projection), then xb rows (gate K/V).
        h1_ctx = ExitStack()
        p_h1 = h1_ctx.enter_context(tc.tile_pool(name="p_h1", bufs=1))
        h1T = p_h1.tile([128, 8, T], BF16, tag="h1T", name="h1T")
        hqT = p_h1.tile([128, 8, 512], BF16, tag="hqT", name="hqT")

        ph1s = h1_ctx.enter_context(tc.tile_pool(name="p_h1s", bufs=3))
        ps_t = h1_ctx.enter_context(tc.tile_pool(name="ps_t", bufs=3, space="PSUM"))

        mv_q, rstd_q = ln_stats(ph1s, [xq_sb[:, j, :] for j in range(NSLOT)], "q")
        for j in range(NSLOT):
            h_rows = ph1s.tile([128, C], BF16, tag="h_rows", name="h_rows")
            ln_apply(xq_sb[:, j, :], mv_q, rstd_q, j, h_rows)
            transpose_block(ps_t, ph1s, h_rows, hqT, j * 128)

        qT = p_att.tile([128, 8, 512], BF16, tag="qT", name="qT")
        kT = p_att.tile([128, 8, T], BF16, tag="kT", name="kT")
        vaug = p_att.tile([128, 8, 16, 65], BF16, tag="vaug", name="vaug")
        yT = p_att.tile([128, 8, 512], BF16, tag="yT", name="yT")

        wq_sb = p_w.tile([128, 8, C], BF16, tag="wslab", name="wslab")
        nc.sync.dma_start(out=wq_sb, in_=wq.rearrange("(a p) c -> p a c", p=128))
        wk_sb = p_w.tile([128, 8, C], BF16, tag="wslab", name="wslab")
        nc.sync.dma_start(out=wk_sb, in_=wk.rearrange("(a p) c -> p a c", p=128))

        ps_mm = h1_ctx.enter_context(tc.tile_pool(name="ps_mm", bufs=4, space="PSUM"))

        # Q^T from hqT -> [C, 512] (evacuate + bias on ScalarE)
        for co in range(8):
            ps = ps_mm.tile([128, 512], F32, tag="mm", name="mm")
            for ci in range(8):
                nc.tensor.matmul(ps, lhsT=wq_sb[:, ci, co * 128:(co + 1) * 128],
                                 rhs=hqT[:, ci, :], start=(ci == 0), stop=(ci == 7))
            nc.scalar.activation(out=qT[:, co, :], in_=ps, func=AF.Identity,
                                 bias=bqc[:, co:co + 1])

        # LN1 of the full batch rows (DVE work overlaps the Q matmuls above)
        x_ts = []
        for r in range(NB):
            x_t = ph1s.tile([128, C], BF16, tag="x_t", name="x_t", bufs=NB)
            nc.sync.dma_start(out=x_t, in_=xb[r * 128:(r + 1) * 128, :])
            x_ts.append(x_t)
        mv_b, rstd_b = ln_stats(ph1s, x_ts, "b")
        for r in range(NB):
            h_rows = ph1s.tile([128, C], BF16, tag="h_rows", name="h_rows")
            ln_apply(x_ts[r], mv_b, rstd_b, r, h_rows)
            transpose_block(ps_t, ph1s, h_rows, h1T, r * 128)

        # K^T from h1T -> [C, T]
        for co in range(8):
            for nt in range(2):
                ps = ps_mm.tile([128, 512], F32, tag="mm", name="mm")
                for ci in range(8):
                    nc.tensor.matmul(
                        ps, lhsT=wk_sb[:, ci, co * 128:(co + 1) * 128],
                        rhs=h1T[:, ci, nt * 512:(nt + 1) * 512],
                        start=(ci == 0), stop=(ci == 7))
                nc.scalar.activation(out=kT[:, co, nt * 512:(nt + 1) * 512], in_=ps,
                                     func=AF.Identity, bias=bkc[:, co:co + 1])

        wv_sb = p_w.tile([128, 8, C], BF16, tag="wslab", name="wslab")
        nc.sync.dma_start(out=wv_sb, in_=wv.rearrange("(a p) c -> p a c", p=128))
        # V rows (bias folded into bo on host), interleaved with ones column
        nc.vector.memset(vaug[:, :, :, 64:65], 1.0)
        for tk in range(8):
            for nt in range(2):
                ps = ps_mm.tile([128, 512], F32, tag="mm", name="mm")
                for ci in range(8):
                    nc.tensor.matmul(
                        ps, lhsT=h1T[:, ci, tk * 128:(tk + 1) * 128],
                        rhs=wv_sb[:, ci, nt * 512:(nt + 1) * 512],
                        start=(ci == 0), stop=(ci == 7))
                nc.scalar.activation(
                    out=vaug[:, tk, nt * 8:(nt + 1) * 8, 0:64],
                    in_=ps.rearrange("p (h d) -> p h d", d=64), func=AF.Identity)

        wo_sb = p_w.tile([128, 8, C], BF16, tag="wslab", name="wslab")
        nc.sync.dma_start(out=wo_sb, in_=wo.rearrange("(a p) c -> p a c", p=128))

        mask_sb = p_att.tile([128, 8, 1024], BF16, tag="masks", name="masks")
        nc.sync.dma_start(out=mask_sb, in_=masks.rearrange("m p q -> p m q"))
        BO = load_bcast(bo, "BO")

        h1_ctx.close()

        # ==== phase 4: attention (pipelined over (slot, head-group)) ========
        mask_idx = {sk: i for i, sk in enumerate(MASKED)}
        groups = [(j, hg) for j in range(NSLOT) for hg in range(2)]

        with tc.tile_pool(name="p_exp", bufs=1) as pexp, \
             tc.tile_pool(name="p_dn", bufs=2) as pdn, \
             tc.tile_pool(name="ps_s", bufs=2, space="PSUM") as ps_s, \
             tc.tile_pool(name="ps_y", bufs=2, space="PSUM") as ps_y:

            def emit_scores(j, hg, kb):
                s_ps = ps_s.tile([128, 1024], F32, tag="s_ps", name="s_ps")
                for p in range(4):
                    hp = 4 * hg + p
                    for hh in range(2):
                        fl = 4 * hh + p
                        nc.tensor.matmul(
                            s_ps[:, fl * 128:(fl + 1) * 128],
                            lhsT=kT[hh * 64:(hh + 1) * 64, hp, kb * 128:(kb + 1) * 128],
                            rhs=qT[hh * 64:(hh + 1) * 64, hp, j * 128:(j + 1) * 128],
                            start=True, stop=True, tile_position=(64 * hh, 0))
                return s_ps

            def emit_exp(j, hg, kb, s_ps):
                expS = pexp.tile([128, 1024], BF16, tag="expS", name="expS", bufs=16)
                nc.scalar.activation(out=expS, in_=s_ps, func=AF.Exp, scale=0.125)
                if (j, kb) in mask_idx:
                    mi = mask_idx[(j, kb)]
                    nc.vector.tensor_mul(out=expS, in0=expS, in1=mask_sb[:, mi, :])
                return expS

            def av_mms(j, hg, expS_list, yaug):
                km = KMAX[j]
                mms = []
                for h8 in range(8):
                    fl = 4 * (h8 % 2) + h8 // 2
                    for kb in range(km):
                        mms.append((yaug[:, h8 * 128:(h8 + 1) * 128],
                                    (kb, 8 * hg + h8),
                                    expS_list[kb][:, fl * 128:(fl + 1) * 128],
                                    kb == 0, kb == km - 1))
                return mms

            def emit_denorm(j, hg, yaug):
                # 1/d via Exp(-Ln(d)) on ScalarE: a [1,N] DVE reciprocal runs
                # single-lane at ~6.4ns/elem (6.5us/call); the two wide ACT
                # calls cost 2.3us and Ln+Exp share one activation table set.
                lnd = pdn.tile([1, 1024], F32, tag="lnd", name="lnd")
                nc.scalar.activation(out=lnd, in_=yaug[64:65, :], func=AF.Ln)
                rbf = pdn.tile([1, 1024], BF16, tag="rbf", name="rbf")
                nc.scalar.activation(out=rbf, in_=lnd, func=AF.Exp, scale=-1.0)
                # K=1 ones-matmul broadcast across 64 partitions; the PSUM
                # tile reuses the scores ring (same shape/tag) to stay in the
                # 8-bank budget
                rb_ps = ps_s.tile([128, 1024], F32, tag="s_ps", name="rb_ps")
                for nt in range(2):
                    nc.tensor.matmul(rb_ps[0:64, nt * 512:(nt + 1) * 512],
                                     lhsT=ones_row, rhs=rbf[:, nt * 512:(nt + 1) * 512],
                                     start=True, stop=True)
                rb_sb = pdn.tile([64, 1024], F32, tag="rb_sb", name="rb_sb")
                nc.vector.tensor_copy(out=rb_sb, in_=rb_ps[0:64, :])
                ya = yaug.rearrange("p (hp two q) -> p hp two q", two=2, q=128)
                rb = rb_sb.rearrange("p (hp two q) -> p hp two q", two=2, q=128)
                for par in range(2):
                    nc.vector.tensor_mul(
                        out=yT[par * 64:(par + 1) * 64, 4 * hg:4 * hg + 4,
                               j * 128:(j + 1) * 128],
                        in0=ya[0:64, :, par, :], in1=rb[0:64, :, par, :])

            prev = None  # (j, hg, pending AV mm list, yaug)
            for j, hg in groups:
                km = KMAX[j]
                expS_list = []
                # split prev group's AV matmuls into km+1 chunks interleaved
                # between this group's score matmuls (keeps PE dense while
                # ScalarE runs the exps)
                if prev is not None:
                    pmms = prev[2]
                    csz = max(1, -(-len(pmms) // (km + 1)))
                    chunks = [pmms[i:i + csz] for i in range(0, len(pmms), csz)]
                else:
                    chunks = []

                def emit_av_chunk(i):
                    if i < len(chunks):
                        for o, (kb, h), e, st, sp in chunks[i]:
                            nc.tensor.matmul(o, lhsT=vaug[:, kb, h, :], rhs=e,
                                             start=st, stop=sp)

                if prev is not None:
                    emit_denorm_prev = lambda: emit_denorm(prev[0], prev[1], prev[3])
                else:
                    emit_denorm_prev = lambda: None

                for kb in range(km):
                    s_ps = emit_scores(j, hg, kb)
                    emit_av_chunk(kb)
                    expS_list.append(emit_exp(j, hg, kb, s_ps))
                for i in range(km, len(chunks)):
                    emit_av_chunk(i)
                emit_denorm_prev()

                yaug = ps_y.tile([65, 1024], F32, tag="yaug", name="yaug")
                prev = (j, hg, av_mms(j, hg, expS_list, yaug), yaug)

            # drain the last group
            for o, (kb, h), e, st, sp in prev[2]:
                nc.tensor.matmul(o, lhsT=vaug[:, kb, h, :], rhs=e, start=st, stop=sp)
            emit_denorm(prev[0], prev[1], prev[3])

        # ==== phase 5: output projection + residual ========================
        with tc.tile_pool(name="ps_pr", bufs=4, space="PSUM") as ps_pr:
            for j in range(NSLOT):
                for nt in range(2):
                    ps = ps_pr.tile([128, 512], F32, tag="prj", name="prj")
                    for ci in range(8):
                        nc.tensor.matmul(
                            ps, lhsT=yT[:, ci, j * 128:(j + 1) * 128],
                            rhs=wo_sb[:, ci, nt * 512:(nt + 1) * 512],
                            start=(ci == 0), stop=(ci == 7))
                    t1 = small.tile([128, 512], F32, tag="prt", name="prt", bufs=2)
                    nc.vector.tensor_add(t1, ps, BO[:, nt * 512:(nt + 1) * 512])
                    nc.vector.tensor_add(
                        xmid[:, j, nt * 512:(nt + 1) * 512], t1,
                        xq_sb[:, j, nt * 512:(nt + 1) * 512])

        att_ctx.close()

        p_mlp = ctx.enter_context(tc.tile_pool(name="p_mlp", bufs=1))
        pw1 = ctx.enter_context(tc.tile_pool(name="p_w1", bufs=2))
        pw2 = ctx.enter_context(tc.tile_pool(name="p_w2", bufs=2))

        b1c = load_cols(b1, 32, "b1c")
        B2 = load_bcast(b2, "B2")

        # prefetch MLP weights (DMA runs under LN2/MLP1 compute)
        w1c = [pw1.tile([128, 8, C], BF16, tag="w1c", name="w1c") for _ in range(2)]
        for chunk in range(2):
            nc.sync.dma_start(
                out=w1c[chunk], in_=w1[:, chunk * C:(chunk + 1) * C]
                .rearrange("(a p) c -> p a c", p=128))
        w2h = [pw2.tile([128, 16, C], BF16, tag="w2h", name="w2h") for _ in range(2)]
        for half in range(2):
            nc.sync.dma_start(
                out=w2h[half], in_=w2[half * 2048:(half + 1) * 2048, :]
                .rearrange("(a p) c -> p a c", p=128))

        # ==== phase 6: LN2 + transpose -> h2T [C, 512] bf16 =================
        h2T = p_mlp.tile([128, 8, 512], BF16, tag="h2T", name="h2T")
        with tc.tile_pool(name="p_h2s", bufs=2) as ph2s, \
             tc.tile_pool(name="ps_t2", bufs=2, space="PSUM") as ps_t2:
            mv_2, rstd_2 = ln_stats(ph2s, [xmid[:, j, :] for j in range(NSLOT)], "2")
            for j in range(NSLOT):
                h2_rows = ph2s.tile([128, C], BF16, tag="h2_rows", name="h2_rows")
                ln_apply(xmid[:, j, :], mv_2, rstd_2, j, h2_rows)
                transpose_block(ps_t2, ph2s, h2_rows, h2T, j * 128)

        # ==== phase 7: MLP1 + gelu -> mT [F, 512] bf16 ======================
        mT = p_mlp.tile([128, 32, 512], BF16, tag="mT", name="mT")
        with tc.tile_pool(name="ps_m1", bufs=4, space="PSUM") as ps_m1:
            for chunk in range(4):
                if chunk >= 2:
                    wc = pw1.tile([128, 8, C], BF16, tag="w1c", name="w1c")
                    nc.sync.dma_start(
                        out=wc, in_=w1[:, chunk * C:(chunk + 1) * C]
                        .rearrange("(a p) c -> p a c", p=128))
                else:
                    wc = w1c[chunk]
                for co8 in range(8):
                    co = chunk * 8 + co8
                    ps = ps_m1.tile([128, 512], F32, tag="m1", name="m1")
                    for ci in range(8):
                        nc.tensor.matmul(
                            ps, lhsT=wc[:, ci, co8 * 128:(co8 + 1) * 128],
                            rhs=h2T[:, ci, :], start=(ci == 0), stop=(ci == 7))
                    nc.scalar.activation(out=mT[:, co, :], in_=ps, func=AF.Gelu,
                                         bias=b1c[:, co:co + 1])

        # ==== phase 8: MLP2 + residual -> out ===============================
        with tc.tile_pool(name="p_out", bufs=2) as pout, \
             tc.tile_pool(name="ps_m2", bufs=8, space="PSUM") as ps_m2:
            pss = [ps_m2.tile([128, 512], F32, tag="m2", name="m2")
                   for _ in range(8)]
            for half in range(2):
                for j in range(NSLOT):
                    for nt in range(2):
                        ps = pss[j * 2 + nt]
                        for ka in range(16):
                            ki = half * 16 + ka
                            nc.tensor.matmul(
                                ps, lhsT=mT[:, ki, j * 128:(j + 1) * 128],
                                rhs=w2h[half][:, ka, nt * 512:(nt + 1) * 512],
                                start=(ki == 0), stop=(ki == 31))
            for j in range(NSLOT):
                o_sb = pout.tile([128, C], F32, tag="o_sb", name="o_sb")
                for nt in range(2):
                    t1 = small.tile([128, 512], F32, tag="ot", name="ot", bufs=2)
                    nc.vector.tensor_add(t1, pss[j * 2 + nt],
                                         B2[:, nt * 512:(nt + 1) * 512])
                    nc.vector.tensor_add(
                        o_sb[:, nt * 512:(nt + 1) * 512], t1,
                        xmid[:, j, nt * 512:(nt + 1) * 512])
                nc.sync.dma_start(out=out[j * 128:(j + 1) * 128, :], in_=o_sb)

    _split_excess_waits(nc)
    return nc


def _split_excess_waits(nc, max_waits=1):
    """walrus rejects engine instructions with >1 sync wait. Hoist excess
    waits onto standalone EventSemaphore (pure-wait) instructions inserted
    just before the offending instruction on the same engine."""
    counter = 0
    for fn in nc.m.functions:
        for bb in fn.blocks:
            insts = bb.instructions
            i = 0
            while i < len(insts):
                inst = insts[i]
                si = getattr(inst, "sync_info", None)
                if os.environ.get("KEEP_DMA_WAITS") and \
                        type(inst).__name__ == "InstDMACopy":
                    i += 1
                    continue
                if (si is not None and si.on_wait
                        and len(si.on_wait) > max_waits):
                    waits = list(si.on_wait)
                    keep, extra = waits[-max_waits:], waits[:-max_waits]
                    for w in extra:
                        ev = mybir.InstEventSemaphore(
                            name=f"splitwait_{counter}", ins=[], outs=[])
                        counter += 1
                        ev.engine = inst.engine
                        ev.bass_nofuse = True
                        ev.sync_info = mybir.SyncInfo(on_wait=[w], on_update=[])
                        nc.register_instruction(ev)
                        insts.insert(i, ev)
                        i += 1
                    inst.sync_info = mybir.SyncInfo(
                        on_wait=keep, on_update=list(si.on_update))
                i += 1


_NC_CACHE = None


def _get_nc():
    global _NC_CACHE
    if _NC_CACHE is None:
        _NC_CACHE = build_nc()
    return _NC_CACHE


def make_masks(parity: int) -> np.ndarray:
    """[8,128,1024] multiplicative bf16 0/1 mask tiles (replicated across the
    8 head-slices) for the MASKED (slot,kb) pairs. Layout [k, q]: keep k<=q."""
    tiles = np.zeros((8, 128, 1024), np.float32)
    tri = (np.arange(128)[:, None] <= np.arange(128)[None, :]).astype(np.float32)
    for i, (slot, kb) in enumerate(MASKED):
        g = QBLOCKS[parity][slot]
        if kb < g:
            tiles[i] = 1.0
        elif kb == g:
            tiles[i] = np.tile(tri, (1, 8))
        else:
            tiles[i] = 0.0
    return tiles.astype(ml_dtypes.bfloat16)


def fold_weights(weights: dict) -> dict:
    """Fold LN gamma/beta into the adjacent projection weights (fp64 on host):
    q = n1 @ (g1*wq) + (bq + b1*wq), same for k; v loses its bias entirely
    (A rows sum to 1 -> bv' routes through wo into bo); ln2 folds into w1."""
    f8 = lambda a: np.asarray(a, np.float64)
    g1, b1g = f8(weights["ln1_g"]), f8(weights["ln1_b"])
    g2, b2g = f8(weights["ln2_g"]), f8(weights["ln2_b"])
    wq, wk, wv, wo = (f8(weights[k]) for k in ("wq", "wk", "wv", "wo"))
    w1, w2 = f8(weights["w1"]), f8(weights["w2"])
    bq, bk, bv, bo = (f8(weights[k]) for k in ("bq", "bk", "bv", "bo"))
    b1, b2 = f8(weights["b1"]), f8(weights["b2"])

    wqf = g1[:, None] * wq
    wkf = g1[:, None] * wk
    wvf = g1[:, None] * wv
    bqf = bq + b1g @ wq
    bkf = bk + b1g @ wk
    bvf = bv + b1g @ wv
    bof = bo + bvf @ wo
    w1f = g2[:, None] * w1
    b1f = b1 + b2g @ w1

    bf = lambda a: np.ascontiguousarray(a.astype(np.float32)).astype(ml_dtypes.bfloat16)
    f32 = lambda a: np.ascontiguousarray(a.astype(np.float32))
    return {
        "wq": bf(wqf), "wk": bf(wkf), "wv": bf(wvf), "wo": bf(wo),
        "w1": bf(w1f), "w2": bf(w2),
        "bq": f32(bqf), "bk": f32(bkf), "bo": f32(bof),
        "b1": f32(b1f), "b2": f32(b2),
    }


def make_in_maps(x: np.ndarray, weights: dict) -> list[dict]:
    f32 = lambda a: np.ascontiguousarray(np.asarray(a, np.float32))
    bf = lambda a: np.ascontiguousarray(np.asarray(a, np.float32)).astype(
        ml_dtypes.bfloat16)
    shared = fold_weights(weights)
    mask_by_parity = [make_masks(0), make_masks(1)]
    in_maps = []
    for core in range(8):
        b, parity = core // 2, core % 2
        qb = QBLOCKS[parity]
        xqg = np.concatenate([x[b, g * 128:(g + 1) * 128, :] for g in qb], axis=0)
        in_maps.append({
            "xb": bf(x[b]), "xq": bf(xqg), "masks": mask_by_parity[parity],
            **shared,
        })
    return in_maps


def assemble_out(results: list[dict]) -> np.ndarray:
    out = np.empty((B, T, C), np.float32)
    for core in range(8):
        b, parity = core // 2, core % 2
        o = np.asarray(results[core]["out"], np.float32)
        for j, g in enumerate(QBLOCKS[parity]):
            out[b, g * 128:(g + 1) * 128, :] = o[j * 128:(j + 1) * 128, :]
    return out


def kernel(**inputs) -> np.ndarray:
    x = np.asarray(inputs["x"], np.float32)
    nc = _get_nc()
    in_maps = make_in_maps(x, inputs)
    res = run_bass_kernel_spmd(nc, in_maps, list(range(8)))
    return assemble_out(res.results)


if __name__ == "__main__":
    _get_nc()
    print("built ok")
